# revision 38
# baseline (speedup 1.0000x reference)
"""Causal GQA self-attention (B=4, T=2048, C=2048, 16 heads / 4 kv-heads,
l2-normalized q,k) on 8 Trainium2 NeuronCores.

Sharding: core pair (2b, 2b+1) handles batch b; parity p takes query rows
p::2 (odd cores receive x with adjacent rows pair-swapped so the fixed
"rows 0::2" gather selects odd rows; the swapped in-chunk key order is
absorbed into mask data).

Because q and k are l2-normalized, |scores| <= 1/sqrt(128), so
exp(s) = 1 + s + O(5e-3) and softmax is near-uniform.  The kernel exploits
this:
  - q/k/v projections run as fp8e4 DoubleRow matmuls (weights pre-scaled
    x32 on the host; the scale cancels in l2norm, and for v it is folded
    into Wproj/32).
  - attention over "full" (strictly-causal) key chunks is linearized:
    sum_k (1+s) v = sum_k v + c * q8^T (K^T V), with K^T V and sum_k k
    prefix states recomputed per 256-query strip (chunked linear
    attention).  The quadratic remainder is O(s^2/2) <= 4e-3 relative on
    near-uniform weights.
  - only the 4 diagonal key chunks per strip get exact exp (fp16) +
    causal-mask multiply + fp16 AV, keeping full accuracy where the
    softmax is sharp (short prefixes).
  - denominators: matmul column-sums (ones weights) of e16 plus the
    linearized ksum^T q8 term; +N_full added on DVE before reciprocal.
"""

from contextlib import ExitStack
from types import SimpleNamespace

import numpy as np
import ml_dtypes

import concourse.bacc as bacc
import concourse.mybir as mybir
import concourse.tile as tile
from concourse.bass_utils import run_bass_kernel_spmd

B, T, C = 4, 2048, 2048
NH, NKV, HD = 16, 4, 128
KV = 512
P = 128
N_CORES = 8

F32 = mybir.dt.float32
BF16 = mybir.dt.bfloat16
FP16 = mybir.dt.float16
FP8 = mybir.dt.float8e4
Exp = mybir.ActivationFunctionType.Exp
Sqrt = mybir.ActivationFunctionType.Sqrt
Copy = mybir.ActivationFunctionType.Copy
MUL = mybir.AluOpType.mult
DR = mybir.MatmulPerfMode.DoubleRow

NST = 4              # 512-token projection strips
NCC = 16             # C contraction chunks of 128
NM_K = KV // P       # 4
NM_Q = C // P        # 16
LQ = T // 2          # 1024 local query rows
NSA = 4              # attention strips
SW = LQ // NSA       # 256 local queries per attention strip

WS = 32.0                                    # host weight pre-scale
C0 = float(1.0 / (1024.0 * np.sqrt(128.0)))  # exp scale / KV16 / ksum scale

# experimental: DoubleRow with 16-bit operands (cost-model 2x; HW-legality
# unverified) — keep False unless the HW run confirms numerics
DR16_OUT = False
DR16_DIAG = False


def _phase_k(nc, tc, g):
    """x transpose, k/v projections + l2norm(k), per 512-token strip."""
    with ExitStack() as ctx:
        p_kT = ctx.enter_context(tc.tile_pool(name="kTb", bufs=1))
        g.kT_sb = p_kT.tile([P, NM_K, T], BF16)  # 16 KB/part, K-phase only
        p_xT = ctx.enter_context(tc.tile_pool(name="xT", bufs=2))
        p_xin = ctx.enter_context(tc.tile_pool(name="xin", bufs=4))
        p_vT = ctx.enter_context(tc.tile_pool(name="vT", bufs=1))
        p_wt = ctx.enter_context(tc.tile_pool(name="wt", bufs=4))
        p_nrm = ctx.enter_context(tc.tile_pool(name="nrm", bufs=2))
        p_sq = ctx.enter_context(tc.tile_pool(name="sq", bufs=2))
        p_stg = ctx.enter_context(tc.tile_pool(name="stg", bufs=2))
        ps_t = ctx.enter_context(tc.tile_pool(name="pt", bufs=3,
                                              space="PSUM"))
        ps_a = ctx.enter_context(tc.tile_pool(name="acc", bufs=3,
                                              space="PSUM"))
        ps_q = ctx.enter_context(tc.tile_pool(name="pssq", bufs=2,
                                              space="PSUM"))
        for st in range(NST):
            _strip_k(nc, g, st, p_xT, p_xin, p_vT, p_wt, p_nrm, p_sq, p_stg,
                     ps_t, ps_a, ps_q)


def _strip_k(nc, g, st, p_xT, p_xin, p_vT, p_wt, p_nrm, p_sq, p_stg,
             ps_t, ps_a, ps_q):
    t0 = st * 512
    tsl = slice(t0, t0 + 512)
    # xT[:, cc, ts, :] = x[t0+ts*128.., cc*128..].T  (fp8 for q/k DR,
    # bf16 for the v projection: fp8 x noise does not average out in y)
    xT = p_xT.tile([P, NCC, NST, P], FP8, tag="xT")
    xT16 = p_xT.tile([P, NCC, NST, P], BF16, tag="xT16")
    for ts in range(4):
        x_sb = p_xin.tile([P, C], BF16, tag="xin")
        nc.sync.dma_start(x_sb[:], g.xb[t0 + ts * P: t0 + (ts + 1) * P, :])
        for c4 in range(4):
            pt = ps_t.tile([P, 4, P], BF16, tag="pt")
            for j in range(4):
                cc = c4 * 4 + j
                nc.tensor.transpose(pt[:, j, :],
                                    x_sb[:, cc * P:(cc + 1) * P], g.ident[:])
            dst = xT[:, c4 * 4:(c4 + 1) * 4, ts, :]
            dst16 = xT16[:, c4 * 4:(c4 + 1) * 4, ts, :]
            if c4 % 2 == 0:
                nc.scalar.copy(out=dst, in_=pt[:])
                nc.vector.tensor_copy(dst16, pt[:])
            else:
                nc.vector.tensor_copy(dst, pt[:])
                nc.scalar.copy(out=dst16, in_=pt[:])
    # strided query-row extraction -> xqh (SBUF->SBUF DMA)
    stg = p_stg.tile([P, NCC, NST, 64], FP8, tag="stg")
    for cc in range(NCC):
        nc.gpsimd.tensor_copy(stg[:, cc], xT[:, cc, :, 0::2])
    nc.sync.dma_start(g.xqh[:, :, st * 256:(st + 1) * 256], stg[:])

    # ---- k projection (fp8 DR) + l2norm ----
    ssqk = ps_q.tile([P, 512], F32, tag="ssq")
    for m in range(NM_K):
        wk_t = p_wt.tile([P, NCC, P], FP8, tag="wt")
        nc.sync.dma_start(wk_t[:], g.wk[m])
        pk = ps_a.tile([P, 512], F32, tag="acc")
        for c in range(8):
            nc.tensor.matmul(pk[:], wk_t[:, 2 * c:2 * c + 2, :],
                             xT[:, 2 * c:2 * c + 2, :, :],
                             start=(c == 0), stop=(c == 7), perf_mode=DR)
        nc.scalar.copy(out=g.kT_sb[:, m, tsl], in_=pk[:])
        sq = p_sq.tile([P, 512], BF16, tag="sq")
        nc.vector.tensor_tensor(sq[:], g.kT_sb[:, m, tsl], g.kT_sb[:, m, tsl],
                                MUL)
        nc.tensor.matmul(ssqk[:], g.ones_bf[:], sq[:],
                         start=(m == 0), stop=(m == NM_K - 1))
    nrm = p_nrm.tile([P, 512], F32, tag="nrm")
    nc.scalar.activation(nrm[:], ssqk[:], Sqrt, scale=float(2.0 ** -10))
    rk = p_nrm.tile([P, 512], F32, tag="rk")
    nc.vector.reciprocal_approx_fast(rk[:], nrm[:])
    rk16 = p_sq.tile([P, 512], BF16, tag="rk16")
    nc.vector.tensor_copy(rk16[:], rk[:])
    nc.vector.tensor_tensor(g.kT_sb[:, :, tsl], g.kT_sb[:, :, tsl],
                            rk16[:, None, :].to_broadcast([P, NM_K, 512]),
                            MUL)
    # fp8 copy + token-major transposes of normalized k
    nc.vector.tensor_copy(g.k8[:, :, tsl], g.kT_sb[:, :, tsl])
    for m in range(NM_K):
        ptk = ps_t.tile([P, 4, P], BF16, tag="pt")
        for j in range(4):
            nc.tensor.transpose(
                ptk[:, j, :], g.kT_sb[:, m, t0 + j * P: t0 + (j + 1) * P],
                g.ident[:])
        nc.vector.tensor_copy(g.kTok[:, st * 4:(st + 1) * 4, m, :], ptk[:])

    # ---- v projection (fp8 DR), transpose to token-major ----
    vT = p_vT.tile([P, NM_K, 512], BF16, tag="vT")
    for m in range(NM_K):
        wv_t = p_wt.tile([P, NCC, P], BF16, tag="wtv")
        nc.sync.dma_start(wv_t[:], g.wv[m])
        pv = ps_a.tile([P, 512], F32, tag="acc")
        for cc in range(NCC):
            nc.tensor.matmul(pv[:], wv_t[:, cc, :], xT16[:, cc, :, :],
                             start=(cc == 0), stop=(cc == NCC - 1))
        nc.scalar.copy(out=vT[:, m, :], in_=pv[:])
    for m in range(NM_K):
        ptv = ps_t.tile([P, 4, P], BF16, tag="pt")
        for j in range(4):
            nc.tensor.transpose(ptv[:, j, :], vT[:, m, j * P:(j + 1) * P],
                                g.ident[:])
        nc.vector.tensor_copy(
            g.v16[:, st * 4:(st + 1) * 4, m * P:(m + 1) * P], ptv[:])


def _phase_q_body(nc, tc, g):
    """q projection (fp8 DR) + l2norm over all 16 chunks."""
    with ExitStack() as ctx:
        p_qT = ctx.enter_context(tc.tile_pool(name="qTb", bufs=1))
        p_wtq = ctx.enter_context(tc.tile_pool(name="wtq", bufs=4))
        p_nrmq = ctx.enter_context(tc.tile_pool(name="nrmq", bufs=2))
        p_sqq = ctx.enter_context(tc.tile_pool(name="sqq", bufs=2))
        ps_aq = ctx.enter_context(tc.tile_pool(name="accq", bufs=3,
                                               space="PSUM"))
        ps_qq = ctx.enter_context(tc.tile_pool(name="ssqq", bufs=2,
                                               space="PSUM"))
        qT_sb = p_qT.tile([P, NM_Q, LQ], BF16)
        ssq = [ps_qq.tile([P, 512], F32, tag="ssq", name=f"ssq{i}")
               for i in range(2)]
        for m in range(NM_Q):
            wq_t = p_wtq.tile([P, NCC, P], FP8, tag="wt")
            nc.sync.dma_start(wq_t[:], g.wq[m])
            for sp in range(2):
                qsl = slice(sp * 512, (sp + 1) * 512)
                pq = ps_aq.tile([P, 512], F32, tag="acc")
                for c in range(8):
                    nc.tensor.matmul(pq[:], wq_t[:, 2 * c:2 * c + 2, :],
                                     g.xqh[:, 2 * c:2 * c + 2, qsl],
                                     start=(c == 0), stop=(c == 7),
                                     perf_mode=DR)
                nc.scalar.copy(out=qT_sb[:, m, qsl], in_=pq[:])
                sq = p_sqq.tile([P, 512], BF16, tag="sq")
                nc.vector.tensor_tensor(sq[:], qT_sb[:, m, qsl],
                                        qT_sb[:, m, qsl], MUL)
                nc.tensor.matmul(ssq[sp][:], g.ones_bf[:], sq[:],
                                 start=(m == 0), stop=(m == NM_Q - 1))
        for sp in range(2):
            qsl = slice(sp * 512, (sp + 1) * 512)
            nrm = p_nrmq.tile([P, 512], F32, tag="nrm")
            nc.scalar.activation(nrm[:], ssq[sp][:], Sqrt,
                                 scale=float(2.0 ** -10))
            rq = p_nrmq.tile([P, 512], F32, tag="rq")
            nc.vector.reciprocal_approx_fast(rq[:], nrm[:])
            rq16 = p_sqq.tile([P, 512], BF16, tag="rq16")
            nc.vector.tensor_copy(rq16[:], rq[:])
            nc.vector.tensor_tensor(qT_sb[:, :, qsl], qT_sb[:, :, qsl],
                                    rq16[:, None, :].to_broadcast(
                                        [P, NM_Q, 512]), MUL)
            nc.vector.tensor_copy(g.q8[:, :, qsl], qT_sb[:, :, qsl])


def _phase_a(nc, tc, g):
    """attention: linear full chunks + exact diagonal, per 256-query strip."""
    with ExitStack() as ctx:
        p_e = ctx.enter_context(tc.tile_pool(name="e", bufs=4))
        p_kvs = ctx.enter_context(tc.tile_pool(name="kvs", bufs=2))
        p_rd = ctx.enter_context(tc.tile_pool(name="rd", bufs=3))
        ps_s = ctx.enter_context(tc.tile_pool(name="ps_s", bufs=2,
                                              space="PSUM"))
        ps_yd = ctx.enter_context(tc.tile_pool(name="ps_yd", bufs=2,
                                               space="PSUM"))
        ps_kv = ctx.enter_context(tc.tile_pool(name="ps_kv", bufs=1,
                                               space="PSUM"))
        for s in range(NSA):
            kv16 = krep = None
            if s > 0:
                kv16, krep = _kv_prefix(nc, g, s, p_kvs, ps_kv)
            for h in range(NH):
                _attn_head(nc, g, s, h, kv16, krep, g.yT, p_e, p_rd, ps_s,
                           ps_yd)


def _kv_prefix(nc, g, s, p_kvs, ps_kv):
    """prefix K^T V and ksum per kv-head (linear branch), strictly causal."""
    kv16 = p_kvs.tile([P, NKV, P], BF16, tag="kv")
    krep = p_kvs.tile([P, NKV, P], BF16, tag="krep")
    ks = p_kvs.tile([P, NKV, 1], F32, tag="ks")
    pkv = ps_kv.tile([P, NKV, P], F32, tag="pkv")
    nck = 4 * s
    for gi in range(NKV):
        gsl = slice(gi * P, (gi + 1) * P)
        for c in range(nck):
            nc.tensor.matmul(pkv[:, gi, :], g.kTok[:, c, gi, :],
                             g.v16[:, c, gsl],
                             start=(c == 0), stop=(c == nck - 1))
        # ksum[hd'] = sum_k khat (x32): free-dim reduction of k8 prefix
        nc.vector.tensor_reduce(ks[:, gi, :], g.k8[:, gi, 0:512 * s],
                                mybir.AxisListType.X, mybir.AluOpType.add)
    nc.scalar.activation(kv16[:], pkv[:], Copy, scale=C0)
    for gi in range(NKV):
        nc.vector.tensor_scalar_mul(krep[:, gi, :],
                                    ks[:, gi, 0:1].to_broadcast([P, P]), C0)
    return kv16, krep


def _attn_head(nc, g, s, h, kv16, krep, yT, p_e, p_rd, ps_s, ps_yd):
    gi = h // 4
    gsl = slice(gi * P, (gi + 1) * P)
    lsl = slice(s * SW, (s + 1) * SW)
    kc0 = 4 * s
    npair = 2 * s
    # diagonal scores (fp8 DoubleRow over hd halves)
    psc = ps_s.tile([P, 4, SW], F32, tag="s")
    for kc in range(4):
        ksl = slice((kc0 + kc) * P, (kc0 + kc + 1) * P)
        nc.tensor.matmul(psc[:, kc, :], g.k8r[:, :, gi, ksl],
                         g.q8r[:, :, h, lsl],
                         start=True, stop=True, perf_mode=DR)
    e16 = p_e.tile([P, 4, SW], FP16, tag="e")
    nc.scalar.activation(e16[:], psc[:], Exp, scale=C0)
    nc.vector.tensor_tensor(e16[:], e16[:], g.mask_sb[:], MUL)

    pyd = ps_yd.tile([P, 2, SW], F32, tag="yd")
    py = pyd[:, 0, :]
    pden = pyd[:, 1, :]
    # numerator chain: sum_full v  +  c*q8^T K^T V  +  diag e16 @ v16
    # (sum_full v MUST use fp16 v: y ~ mean(v), so fp8 noise on v would not
    # average down relative to y)
    first = True
    for c in range(4 * s):
        nc.tensor.matmul(py, g.v16[:, c, gsl], g.ones8[:, 0, :],
                         start=first, stop=False)
        first = False
    if s > 0:
        nc.tensor.matmul(py, kv16[:, gi, :], g.q8[:, h, lsl],
                         start=first, stop=False)
        first = False
    if DR16_DIAG:
        for c in range(2):
            nc.tensor.matmul(py, g.v16[:, kc0 + 2 * c:kc0 + 2 * c + 2, gsl],
                             e16[:, 2 * c:2 * c + 2, :],
                             start=first, stop=(c == 1), perf_mode=DR)
            first = False
    else:
        for kc in range(4):
            nc.tensor.matmul(py, g.v16[:, kc0 + kc, gsl], e16[:, kc, :],
                             start=first, stop=(kc == 3))
            first = False
    # denominator chain
    first = True
    if s > 0:
        nc.tensor.matmul(pden, krep[:, gi, :], g.q8[:, h, lsl],
                         start=True, stop=False)
        first = False
    if DR16_DIAG:
        for c in range(2):
            nc.tensor.matmul(pden,
                             g.ones_h[:, None, :].to_broadcast([P, 2, P]),
                             e16[:, 2 * c:2 * c + 2, :],
                             start=first, stop=(c == 1), perf_mode=DR)
            first = False
    else:
        for kc in range(4):
            nc.tensor.matmul(pden, g.ones_h[:], e16[:, kc, :],
                             start=first, stop=(kc == 3))
            first = False
    # rden = 1 / (pden + 512*s); yT = py * rden
    rden = p_rd.tile([P, SW], F32, tag="rd")
    if s > 0:
        tden = p_rd.tile([P, SW], F32, tag="td")
        nc.vector.tensor_scalar_add(tden[:], pden, float(512 * s))
        nc.vector.reciprocal_approx_fast(rden[:], tden[:])
    else:
        nc.vector.reciprocal_approx_fast(rden[:], pden)
    nc.vector.tensor_tensor(yT[:, h, lsl], py, rden[:], MUL)


def _phase_o(nc, tc, g):
    """out-projection: wp loaded once per 256-feature chunk, full LQ."""
    with ExitStack() as ctx:
        p_wp = ctx.enter_context(tc.tile_pool(name="wpt", bufs=2))
        p_o = ctx.enter_context(tc.tile_pool(name="osb", bufs=2))
        ps_o = ctx.enter_context(tc.tile_pool(name="ps_o", bufs=4,
                                              space="PSUM"))
        for og in range(8):
            wp_t = p_wp.tile([P, NH, 2 * P], BF16, tag="wpt")
            nc.sync.dma_start(wp_t[:], g.wp[og])
            o_sb = p_o.tile([P, 2, LQ], F32, tag="o")
            for j in range(2):
                for qh in range(2):
                    qsl = slice(qh * 512, (qh + 1) * 512)
                    po = ps_o.tile([P, 512], F32, tag="po")
                    if DR16_OUT:
                        for c in range(NH // 2):
                            nc.tensor.matmul(
                                po[:],
                                wp_t[:, 2 * c:2 * c + 2, j * P:(j + 1) * P],
                                g.yT[:, 2 * c:2 * c + 2, qsl],
                                start=(c == 0), stop=(c == 7), perf_mode=DR)
                    else:
                        for hh in range(NH):
                            nc.tensor.matmul(
                                po[:], wp_t[:, hh, j * P:(j + 1) * P],
                                g.yT[:, hh, qsl],
                                start=(hh == 0), stop=(hh == NH - 1))
                    nc.vector.tensor_copy(o_sb[:, j, qsl], po[:])
            nc.sync.dma_start(
                g.out.rearrange("(og j p) q -> p og j q", p=P, j=2)[:, og],
                o_sb[:])


def build():
    nc = bacc.Bacc("TRN2", target_bir_lowering=False, debug=False,
                   num_devices=N_CORES)
    g = SimpleNamespace()
    g.xb = nc.declare_dram_parameter("xb", [T, C], BF16, isOutput=False)
    # weights pre-staged on the host into the exact SBUF layouts so every
    # DMA descriptor is a >=2KB contiguous run
    g.wq = nc.declare_dram_parameter("wq", [NM_Q, P, NCC, P], FP8,
                                     isOutput=False)
    g.wk = nc.declare_dram_parameter("wk", [NM_K, P, NCC, P], FP8,
                                     isOutput=False)
    # wv must be bf16: fp8 weight noise is a fixed linear-map error that
    # does not average out in sum_k v (y ~ mean(v))
    g.wv = nc.declare_dram_parameter("wv", [NM_K, P, NCC, P], BF16,
                                     isOutput=False)
    g.wp = nc.declare_dram_parameter("wp", [8, P, NH, 2 * P], BF16,
                                     isOutput=False)
    masks = nc.declare_dram_parameter("masks", [4, P, SW], FP16,
                                      isOutput=False)
    ident_in = nc.declare_dram_parameter("ident", [P, P], BF16, isOutput=False)
    onesb_in = nc.declare_dram_parameter("onesb", [P, P], BF16, isOutput=False)
    onesh_in = nc.declare_dram_parameter("onesh", [P, P], FP16, isOutput=False)
    ones8_in = nc.declare_dram_parameter("ones8", [P, 2, SW], FP8,
                                         isOutput=False)
    g.out = nc.declare_dram_parameter("out", [C, LQ], F32, isOutput=True)

    with tile.TileContext(nc) as tc, ExitStack() as ctx:
        cst = ctx.enter_context(tc.tile_pool(name="cst", bufs=1))
        p_k8 = ctx.enter_context(tc.tile_pool(name="k8p", bufs=1))
        p_ktok = ctx.enter_context(tc.tile_pool(name="ktok", bufs=1))
        p_v16 = ctx.enter_context(tc.tile_pool(name="v16p", bufs=1))
        p_q8 = ctx.enter_context(tc.tile_pool(name="q8p", bufs=1))
        p_xqh = ctx.enter_context(tc.tile_pool(name="xqh", bufs=1))

        g.ident = cst.tile([P, P], BF16)
        nc.sync.dma_start(g.ident[:], ident_in[:])
        g.ones_bf = cst.tile([P, P], BF16)
        nc.sync.dma_start(g.ones_bf[:], onesb_in[:])
        g.ones_h = cst.tile([P, P], FP16)
        nc.sync.dma_start(g.ones_h[:], onesh_in[:])
        g.ones8 = cst.tile([P, 2, SW], FP8)
        nc.sync.dma_start(g.ones8[:], ones8_in[:])
        g.mask_sb = cst.tile([P, 4, SW], FP16)
        nc.sync.dma_start(g.mask_sb[:], masks.rearrange("j p f -> p j f"))

        g.k8 = p_k8.tile([P, NM_K, T], FP8)           # 8 KB
        g.kTok = p_ktok.tile([P, NCC, NM_K, P], FP8)  # 8 KB token-major
        g.v16 = p_v16.tile([P, NCC, KV], FP16)        # 16 KB (32*v)
        g.q8 = p_q8.tile([P, NM_Q, LQ], FP8)          # 16 KB (32*qhat)
        g.xqh = p_xqh.tile([P, NCC, LQ], FP8)         # 16 KB query rows^T

        _phase_k(nc, tc, g)
        _phase_q_body(nc, tc, g)

        with ExitStack() as ctx2:
            p_rep = ctx2.enter_context(tc.tile_pool(name="rep", bufs=1))
            p_yTg = ctx2.enter_context(tc.tile_pool(name="yTg", bufs=1))
            g.q8r = p_rep.tile([64, 2, NM_Q, LQ], FP8)  # 32 KB parts 0..63
            g.k8r = p_rep.tile([64, 2, NM_K, T], FP8)   # 16 KB parts 0..63
            g.yT = p_yTg.tile([P, NH, LQ], BF16)        # 32 KB

            # repack q8/k8 to [64, 2(hd-half), ...] for DoubleRow scores
            nc.sync.dma_start(g.q8r[:, 0], g.q8[0:64])
            nc.sync.dma_start(g.q8r[:, 1], g.q8[64:128])
            nc.sync.dma_start(g.k8r[:, 0], g.k8[0:64])
            nc.sync.dma_start(g.k8r[:, 1], g.k8[64:128])

            _phase_a(nc, tc, g)
            _phase_o(nc, tc, g)

    nc.compile()
    return nc


_NC = None


def _get_nc():
    global _NC
    if _NC is None:
        _NC = build()
    return _NC


def _make_masks(p: int) -> np.ndarray:
    j = np.arange(4)[:, None, None]
    k = np.arange(P)[None, :, None]
    q = np.arange(SW)[None, None, :]
    kk = k if p == 0 else (k ^ 1)
    valid = (2 * q + p) >= (128 * j + kk)
    return valid.astype(np.float16)


def _fp8(a: np.ndarray) -> np.ndarray:
    return np.clip(a, -230.0, 230.0).astype(ml_dtypes.float8_e4m3)


def kernel(x, Wq, Wkv, Wproj):
    x = np.asarray(x, dtype=np.float32)
    Wq = np.asarray(Wq, dtype=np.float32)
    Wkv = np.asarray(Wkv, dtype=np.float32)
    Wproj = np.asarray(Wproj, dtype=np.float32)

    # relayout: w[m, p, cc, f] = W[cc*128+p, m*128+f] (contiguous DMA runs)
    wq8 = np.ascontiguousarray(
        _fp8(WS * Wq).reshape(NCC, P, NM_Q, P).transpose(2, 1, 0, 3))
    wk8 = np.ascontiguousarray(
        _fp8(WS * Wkv[:, :KV]).reshape(NCC, P, NM_K, P).transpose(2, 1, 0, 3))
    wv8 = np.ascontiguousarray(
        (WS * Wkv[:, KV:]).astype(ml_dtypes.bfloat16)
        .reshape(NCC, P, NM_K, P).transpose(2, 1, 0, 3))
    # wp[og, p, hh, f] = (Wproj/32)[hh*128+p, og*256+f]
    wp16 = np.ascontiguousarray(
        (Wproj / WS).astype(ml_dtypes.bfloat16)
        .reshape(NH, P, 8, 2 * P).transpose(2, 1, 0, 3))
    ident = np.eye(P, dtype=ml_dtypes.bfloat16)
    onesb = np.ones((P, P), dtype=ml_dtypes.bfloat16)
    onesh = np.ones((P, P), dtype=np.float16)
    ones8 = np.ones((P, 2, SW), dtype=ml_dtypes.float8_e4m3)
    masks_by_p = [_make_masks(0), _make_masks(1)]

    in_maps = []
    for c in range(N_CORES):
        b, p = c // 2, c % 2
        if p == 0:
            xb_c = x[b].astype(ml_dtypes.bfloat16)
        else:
            xb_c = (x[b].reshape(T // 2, 2, C)[:, ::-1, :]
                    .reshape(T, C).astype(ml_dtypes.bfloat16))
        in_maps.append({
            "xb": np.ascontiguousarray(xb_c),
            "wq": wq8, "wk": wk8, "wv": wv8, "wp": wp16,
            "masks": masks_by_p[p],
            "ident": ident, "onesb": onesb, "onesh": onesh, "ones8": ones8,
        })

    nc = _get_nc()
    res = run_bass_kernel_spmd(nc, in_maps, list(range(N_CORES)),
                               trace=False)

    result = np.empty((B, T, C), dtype=np.float32)
    for c in range(N_CORES):
        b, p = c // 2, c % 2
        result[b, p::2, :] = res.results[c]["out"].T
    return result
